# revision 65
# baseline (speedup 1.0000x reference)
"""Trainium2 Bass kernel for fused LoRA-attention block (nn_Attention_18846316494887).

Reference computation:
  qkv = y @ Wqkv.T + bqkv (+ LoRA deltas y @ (B@A) per Q/K/V)  -> Q,K,V [B,H,S,D]
  attn = softmax(Q K^T / sqrt(D)); o = attn @ V -> [B,S,E]
  msa = o @ Wmsa.T + o @ (Bo@Ao); res = msa + y; out = LayerNorm(res)*gamma + beta

Sharding: tensor-parallel over heads (2 heads/core, 8 cores), AllToAll to
reshard head-dim -> token-dim before the output projection, token-parallel
msa + LayerNorm, host-side gather of per-core token shards.

Precision plan (error budget: attention path contributes only ~2.2% of the
LN'd output norm, so a few-% relative error there is invisible):
  - y, Wqkv (x32), V, exp(scores) all in fp8e4m3; f32 PSUM accumulation
  - Q/K projection matmuls in DoubleRow mode (2 fp8 k-subtiles per pass)
  - AV matmuls in DoubleRow mode over kt-pairs (halves the ex stream time)
  - the x32*x32 weight scaling and 1/sqrt(D) fold into the exp's free
    affine scale (exp(x * 1/8192)); V-scale folds into msa weights (/32)

Host-side prep (exact algebra, no approximation):
  - LoRA folded into Wqkv / Wmsa (y@W.T + y@(B@A) == y@(W.T + B@A))
  - V bias folded into the residual shard: (o+bv)@M + y == o@M + (y + bv@M)
  - y pre-transposed to [E, T] for the QKV matmuls

Schedule (v1): startup DMAs striped over 4 engine queues (first exp ~4us);
softmax finalize normalizes straight from PSUM into the A2A DRAM payload
(recip at psum partition 0 via ones-first V padding); msa+LayerNorm for
shards 0-2 interleave into b1's attention as background PE/DVE work (LN
rstd via poly+Newton on DVE, so the ACT exp table is never swapped);
warm-up dummies keep the PE p-state high over the last collective.
"""
import functools
import numpy as np
import ml_dtypes

import concourse.mybir as mybir
import concourse.tile as tile
from concourse import bacc
from concourse import bass_utils
from concourse.bass import _add_dep_helper

# problem shapes (hardcoded per harness contract)
E = 1024
H = 16
D = 64
B = 2
S = 2048
T = B * S          # 4096 tokens
N_CORES = 8
EPS = 1e-6

BF16 = mybir.dt.bfloat16
F32 = mybir.dt.float32
F8 = mybir.dt.float8e4
NP_F8 = ml_dtypes.float8_e4m3
AF = mybir.ActivationFunctionType
ALU = mybir.AluOpType
DR = mybir.MatmulPerfMode.DoubleRow

# per-core worksizes
TOK = T // N_CORES          # 512 tokens per core for msa/LN
QC = 512                    # attention q-chunk
N_QC = S // QC              # 4 q-chunks per (b, head-pair)
N_KT = S // 128             # 16 k-tiles
N_KP = N_KT // 2            # 8 kt-pairs (DoubleRow AV granularity)
VW = 80                     # padded V row (64 d + 1 ones + pad to 16B mult)
WSC = 32.0                  # fp8 weight pre-scale
S_ACT = 1.0 / (WSC * WSC * 8.0)   # exp affine scale: /32^2 (w-scales) /sqrt(D)

# quadratic seed for 1/sqrt(v) on v in [0.4, 3.0]; 3 Newton steps follow
RSQ_C2 = 0.131712920391688
RSQ_C1 = -0.7403870149754509
RSQ_C0 = 1.6463305621254531


def _build(use_gamma: bool, use_beta: bool):
    nc = bacc.Bacc("TRN2", target_bir_lowering=False, debug=False, num_devices=N_CORES)

    # ---- DRAM parameters -------------------------------------------------
    yT = nc.dram_tensor("yT", [E, T], F8, kind="ExternalInput")
    wqT = nc.dram_tensor("wqT", [E, 128], F8, kind="ExternalInput")
    wkT = nc.dram_tensor("wkT", [E, 128], F8, kind="ExternalInput")
    wvT = nc.dram_tensor("wvT", [E, 128], F8, kind="ExternalInput")
    bq = nc.dram_tensor("bq", [128, 1], F32, kind="ExternalInput")
    msa_w = nc.dram_tensor("msa_w", [E, E], F8, kind="ExternalInput")
    y_shard = nc.dram_tensor("y_shard", [TOK, E], BF16, kind="ExternalInput")
    if use_gamma:
        gamma_b = nc.dram_tensor("gamma_b", [128, E], F32, kind="ExternalInput")
    if use_beta:
        beta_b = nc.dram_tensor("beta_b", [128, E], F32, kind="ExternalInput")
    out = nc.dram_tensor("out", [TOK, E], F32, kind="ExternalOutput")

    # internal DRAM: A2A bounce buffers (shard k: (b, q-half) -> 128 tok/core)
    a2a_in = [nc.dram_tensor(f"a2a_in{k}", [N_CORES, 128, 128], F8) for k in range(4)]
    a2a_out = [nc.dram_tensor(f"a2a_out{k}", [N_CORES, 128, 128], F8) for k in range(4)]

    with tile.TileContext(nc) as tc:
        with (
            tc.tile_pool(name="const", bufs=1) as cpool,
            tc.tile_pool(name="yt", bufs=5) as ytp,
            tc.tile_pool(name="qk", bufs=1) as qkp,
            tc.tile_pool(name="exp", bufs=3) as expp,
            tc.tile_pool(name="stage", bufs=1) as stp,
            tc.tile_pool(name="fin", bufs=2) as finp,
            tc.tile_pool(name="sh", bufs=3) as shp,
            tc.tile_pool(name="a2asb", bufs=2) as a2ap,
            tc.tile_pool(name="ps_acc", bufs=2, space="PSUM") as ps_acc,
            tc.tile_pool(name="ps_sc", bufs=2, space="PSUM") as ps_sc,
            tc.tile_pool(name="ps_av", bufs=2, space="PSUM") as ps_av,
        ):
            # ---- constants + all bulk DMAs, striped over 4 engine queues ----
            wqT_sb = cpool.tile([128, 8, 128], F8)
            wkT_sb = cpool.tile([128, 8, 128], F8)
            wvT_sb = cpool.tile([128, 8, 128], F8)
            nc.sync.dma_start(wqT_sb[:], wqT[:, :].rearrange("(a p) n -> p a n", p=128))
            nc.gpsimd.dma_start(wkT_sb[:], wkT[:, :].rearrange("(a p) n -> p a n", p=128))
            nc.scalar.dma_start(wvT_sb[:], wvT[:, :].rearrange("(a p) n -> p a n", p=128))
            bq_sb = cpool.tile([128, 1], F32)
            nc.scalar.dma_start(bq_sb[:], bq[:, :])
            msa_w_sb = cpool.tile([128, 8, E], F8)
            y_shard_sb = cpool.tile([128, 4, E], BF16)
            if use_gamma:
                gamma_sb = cpool.tile([128, E], F32)
            if use_beta:
                beta_sb = cpool.tile([128, E], F32)

            # b0 chunk0 striped over 3 queues FIRST (so the sem thresholds
            # the framework coalesces for the first K/Q matmuls stay tight);
            # the remaining loads are emitted after the chunk0 projections.
            yts = {0: [], 1: []}
            q3 = [nc.sync, nc.gpsimd, nc.scalar]
            for b in (0, 1):
                for tc8 in range(4):
                    yt = ytp.tile([128, 8, 512], F8, tag="yt", name=f"yt{b}_{tc8}")
                    yts[b].append(yt)

            def emit_y_load(b, tc8, et):
                if b == 1:
                    eng = nc.sync
                elif tc8 == 0:
                    eng = q3[et % 3]
                else:
                    eng = nc.sync if et % 2 == 0 else nc.gpsimd
                eng.dma_start(
                    yts[b][tc8][:, et, :],
                    yT[128 * et:128 * (et + 1),
                       b * S + 512 * tc8: b * S + 512 * (tc8 + 1)])

            for et in range(8):
                emit_y_load(0, 0, et)

            def emit_late_loads():
                for tc8 in (1, 2, 3):
                    for et in range(8):
                        emit_y_load(0, tc8, et)
                for tc8 in range(4):
                    for et in range(8):
                        emit_y_load(1, tc8, et)
            def emit_bulk_consts():
                # warm the gpsimd broadcast library (LOAD_LIB stalls the Q7s
                # ~9us one-time) in the attention window where nothing waits
                # on gpsimd yet; bulk consts after all y tiles on sync
                libw = cpool.tile([1, 16], F32, name="libw")
                libw2 = cpool.tile([2, 16], F32, name="libw2")
                nc.vector.memset(libw[0:1, :], 1.0)
                nc.gpsimd.partition_broadcast(libw2[:, :], libw[0:1, :])
                nc.sync.dma_start(msa_w_sb[:],
                                  msa_w[:, :].rearrange("(a p) n -> p a n", p=128))
                nc.sync.dma_start(y_shard_sb[:],
                                  y_shard[:, :].rearrange("(a p) n -> p a n", p=128))
                if use_gamma:
                    nc.sync.dma_start(gamma_sb[:], gamma_b[:, :])
                if use_beta:
                    nc.sync.dma_start(beta_sb[:], beta_b[:, :])

            # V tiles, padded: [k-part, b, head, kt, VW]; col 64 = ones (o
            # rows land on PSUM 0:64 -- base-0 DVE access -- denom on p64)
            v_sb = cpool.tile([128, B, 2, N_KT, VW], F8)
            nc.vector.memset(v_sb[:, :, :, :, 64:VW], 0.0)
            nc.vector.memset(v_sb[:, :, :, :, 64:65], 1.0)

            # Q^T/K^T in fp8 for DoubleRow scores: [32-row strip per head,
            # 2 k-subtiles, b, tok]; partition p<32 = head0 d=(p + 32*sub),
            # p in 32:64 = head1. The projection writes sub0 lane-aligned;
            # sub1 goes through a small partition-shift DMA.
            qT_sb = qkp.tile([64, 2, B, S], F8)
            kT_sb = qkp.tile([64, 2, B, S], F8)
            # residual rows for the 4 token shards
            res_sb = stp.tile([128, 4, E], F32)
            mu_t = [cpool.tile([128, 6], F32, name=f"mu{k}") for k in range(4)]
            first_scores = []  # scores(0,1) instr, ordering anchor for V-c0
            trigs = []         # collective trigger instrs, k-order
            av_anchors = []    # b1 per-qc last-AV instrs (attention(1) fills)

            # ============== QKV projection step factories ==============
            def make_qkv_steps(b):
                """QKV projection for batch b as small closures so the PE work
                interleaves into attention (fills ACT-wait slots)."""
                qs, ks, vs = [], [], []
                for tc8 in range(4):
                    st8 = {"yt": yts[b][tc8]}
                    qs.append([])
                    ks.append([])
                    vs.append([])

                    # Q/K: 4 DoubleRow matmuls (et-pairs), K=1024 contraction.
                    # PSUM partitions are host-permuted: 0:64 = sub0 dims of
                    # both heads (lane-aligned copy), 64:128 = sub1 dims
                    # (partition-shift DMA into the fp8 qT/kT layout).
                    sl = slice(512 * tc8, 512 * (tc8 + 1))
                    for eg in range(4):
                        def qstep(b=b, tc8=tc8, eg=eg, st8=st8, sl=sl):
                            if eg == 0:
                                st8["ps_q"] = ps_acc.tile([128, 512], F32, tag="acc", name="ps_q")
                            ps_q, yt = st8["ps_q"], st8["yt"]
                            st, sp = (eg == 0), (eg == 3)
                            nc.tensor.matmul(ps_q[:], wqT_sb[:, 2 * eg:2 * eg + 2, :],
                                             yt[:, 2 * eg:2 * eg + 2, :], start=st, stop=sp,
                                             perf_mode=DR)
                            if eg == 3:
                                nc.vector.tensor_scalar(
                                    qT_sb[0:64, 0, b, sl], ps_q[0:64, :],
                                    bq_sb[0:64], None, ALU.add)
                                sh = shp.tile([128, 512], F8, tag="sh")
                                # non-zero-base PSUM reads are capped at 32
                                # partitions -- split the sub1 copy
                                nc.vector.tensor_scalar(
                                    sh[64:96, :], ps_q[64:96, :],
                                    bq_sb[64:96], None, ALU.add)
                                nc.vector.tensor_scalar(
                                    sh[96:128, :], ps_q[96:128, :],
                                    bq_sb[96:128], None, ALU.add)
                                eng = nc.scalar if (b == 0 and tc8 == 0) else nc.gpsimd
                                eng.dma_start(qT_sb[0:64, 1, b, sl], sh[64:128, :])
                        qs[tc8].append(qstep)

                    # K bias is softmax-invariant (adds a per-query constant
                    # to every logit in a row) -- dropped entirely.
                    for eg in range(4):
                        def kstep(b=b, tc8=tc8, eg=eg, st8=st8, sl=sl):
                            if eg == 0:
                                st8["ps_k"] = ps_acc.tile([128, 512], F32, tag="acc", name="ps_k")
                            ps_k, yt = st8["ps_k"], st8["yt"]
                            st, sp = (eg == 0), (eg == 3)
                            mi = nc.tensor.matmul(ps_k[:], wkT_sb[:, 2 * eg:2 * eg + 2, :],
                                                  yt[:, 2 * eg:2 * eg + 2, :], start=st, stop=sp,
                                                  perf_mode=DR)
                            if b == 0 and tc8 == 1 and eg == 0 and first_scores:
                                _add_dep_helper(
                                    mi.ins, first_scores[-1].ins, sync=False,
                                    reason="K-c1 behind first scores")
                            if eg == 3:
                                nc.vector.tensor_copy(kT_sb[0:64, 0, b, sl], ps_k[0:64, :])
                                sh = shp.tile([128, 512], F8, tag="sh")
                                nc.vector.tensor_copy(sh[64:96, :], ps_k[64:96, :])
                                nc.vector.tensor_copy(sh[96:128, :], ps_k[96:128, :])
                                eng = nc.scalar if (b == 0 and tc8 == 0) else nc.gpsimd
                                eng.dma_start(kT_sb[0:64, 1, b, sl], sh[64:128, :])
                        ks[tc8].append(kstep)

                    # V: [tok, vdim] layout, fp8 operands (no DoubleRow: the
                    # stationary operand changes every matmul, FWL covers it)
                    for eg in range(4):
                        def vstep(b=b, tc8=tc8, eg=eg, st8=st8):
                            if eg == 0:
                                st8["ps_v"] = ps_acc.tile([128, 512], F32, tag="acc", name="ps_v")
                            ps_v, yt = st8["ps_v"], st8["yt"]
                            for et in (2 * eg, 2 * eg + 1):
                                st, sp = (et == 0), (et == 7)
                                for s4 in range(4):
                                    mi = nc.tensor.matmul(ps_v[:, 128 * s4:128 * (s4 + 1)],
                                                          yt[:, et, 128 * s4:128 * (s4 + 1)],
                                                          wvT_sb[:, et, :], start=st, stop=sp)
                                    if b == 0 and tc8 == 0 and et == 0 and s4 == 0 \
                                            and first_scores:
                                        # scheduler-ordering hint: keep V
                                        # chunk0 behind the first scores so
                                        # the ACT exp stream starts ~16us
                                        # earlier (V is only needed by av at
                                        # step ~5)
                                        _add_dep_helper(
                                            mi.ins, first_scores[-1].ins, sync=False,
                                            reason="V-c0 behind first scores")
                            if eg == 3:
                                for h in range(2):
                                    src = ps_v[:, :].rearrange(
                                        "p (s n) -> p s n", s=4)[:, :, 64 * h:64 * (h + 1)]
                                    nc.vector.tensor_copy(
                                        v_sb[:, b, h, 4 * tc8:4 * (tc8 + 1), 0:64], src)
                        vs[tc8].append(vstep)
                return qs, ks, vs

            # ============== attention ==============
            def attention(b, bg, av_last=None, qcs=range(N_QC), pops=2):
                # software-pipelined ACROSS kt steps: qk/exp runs OV steps
                # ahead of av; bg closures fill the PE's ACT-wait slots.
                if av_last is None:
                    av_last = []
                OV = 4
                states = {}

                def qk_exp(qc, kt):
                    stq = states[qc]
                    if kt % 2 == 0:
                        stq["exs"][kt // 2] = expp.tile([128, 2, 1024], F8, name="ex")
                    sc = ps_sc.tile([128, 1024], F32, tag="sc", name="sc")
                    ktsl = slice(128 * kt, 128 * (kt + 1))
                    qcsl = slice(QC * qc, QC * (qc + 1))
                    # fp8 DoubleRow scores, two concurrent 32-row strips
                    nc.tensor.matmul(sc[:, 0:512],
                                     kT_sb[0:32, :, b, ktsl],
                                     qT_sb[0:32, :, b, qcsl],
                                     start=True, stop=True, perf_mode=DR,
                                     tile_position=(0, 0))
                    s2 = nc.tensor.matmul(sc[:, 512:1024],
                                          kT_sb[32:64, :, b, ktsl],
                                          qT_sb[32:64, :, b, qcsl],
                                          start=True, stop=True, perf_mode=DR,
                                          tile_position=(32, 0))
                    if b == 0 and qc == 0 and kt == 1:
                        first_scores.append(s2)
                    ex = stq["exs"][kt // 2]
                    nc.scalar.activation(ex[:, kt % 2, :], sc[:], AF.Exp, scale=S_ACT)

                def av_a(qc, kp):
                    stq = states[qc]
                    if kp == 0:
                        stq["av_a"] = ps_av.tile([128, 512], F32, tag="av", name="av_a")
                        stq["av_b"] = ps_av.tile([128, 512], F32, tag="av", name="av_b")
                    ex = stq["exs"][kp]
                    nc.tensor.matmul(stq["av_a"][0:65, :],
                                     v_sb[:, b, 0, 2 * kp:2 * kp + 2, 0:65],
                                     ex[:, :, 0:512],
                                     start=(kp == 0), stop=(kp == N_KP - 1), perf_mode=DR)

                def av_b(qc, kp):
                    stq = states[qc]
                    ex = stq["exs"][kp]
                    i2 = nc.tensor.matmul(stq["av_b"][0:65, :],
                                          v_sb[:, b, 1, 2 * kp:2 * kp + 2, 0:65],
                                          ex[:, :, 512:1024],
                                          start=(kp == 0), stop=(kp == N_KP - 1), perf_mode=DR)
                    if kp == N_KP - 1:
                        av_last.append(i2)

                def finalize(qc):
                    # av psum: partitions 0:64 = o rows, partition 64 = denom.
                    # FAST psum drain to SBUF first (av pool has only 2 slots;
                    # holding psum through the recip chain stalls the next
                    # q-chunk's AV), then denom -> p0, recip, broadcast, one
                    # fused mult straight to fp8, DMA into the A2A payload.
                    av_a, av_b = states[qc]["av_a"], states[qc]["av_b"]
                    af = finp.tile([65, 1024], F32, tag="af", name="af")
                    nc.vector.tensor_copy(af[0:65, 0:512], av_a[0:65, :])
                    nc.vector.tensor_copy(af[0:65, 512:1024], av_b[0:65, :])
                    rc = finp.tile([1, 1024], F32, tag="rc", name="rc")
                    nc.gpsimd.dma_start(rc[0:1, :], af[64:65, :])
                    rc2 = finp.tile([1, 1024], F32, tag="rc2", name="rc2")
                    nc.vector.reciprocal_approx_fast(rc2[0:1, :], rc[0:1, :])
                    rb = finp.tile([64, 1024], F32, tag="rb", name="rb")
                    nc.gpsimd.partition_broadcast(rb[:, :], rc2[0:1, :])
                    ta = finp.tile([64, 512], F8, tag="ta", name="ta")
                    tb = finp.tile([64, 512], F8, tag="tb", name="tb")
                    nc.vector.tensor_tensor(ta[:, :], af[0:64, 0:512],
                                            rb[:, 0:512], ALU.mult)
                    nc.vector.tensor_tensor(tb[:, :], af[0:64, 512:1024],
                                            rb[:, 512:1024], ALU.mult)
                    hf = qc // 2
                    qh = qc % 2
                    k = 2 * b + hf
                    nc.gpsimd.dma_start(
                        a2a_in[k].ap()[4 * qh:4 * qh + 4, 0:64, :].rearrange("j p n -> p j n"),
                        ta[:, :].rearrange("p (j n) -> p j n", j=4))
                    nc.gpsimd.dma_start(
                        a2a_in[k].ap()[4 * qh:4 * qh + 4, 64:128, :].rearrange("j p n -> p j n"),
                        tb[:, :].rearrange("p (j n) -> p j n", j=4))
                    if qh == 1:
                        trigs.append(nc.gpsimd.collective_compute(
                            "AllToAll", ALU.bypass,
                            replica_groups=[list(range(N_CORES))],
                            ins=[a2a_in[k].ap().opt()],
                            outs=[a2a_out[k].ap().opt()],
                        ))

                seq = [(qc, kt) for qc in qcs for kt in range(N_KT)]
                for i, (qc, kt) in enumerate(seq):
                    states.setdefault(qc, {"exs": [None] * N_KP})
                    qk_exp(qc, kt)
                    for _ in range(pops):
                        if bg:
                            bg.pop(0)()
                    j = i - OV
                    if j >= 0 and seq[j][1] % 2 == 1:
                        jqc, jkt = seq[j]
                        av_a(jqc, jkt // 2)
                        av_b(jqc, jkt // 2)
                        if jkt == N_KT - 1:
                            finalize(jqc)
                for j in range(max(0, len(seq) - OV), len(seq)):
                    if seq[j][1] % 2 == 1:
                        jqc, jkt = seq[j]
                        av_a(jqc, jkt // 2)
                        av_b(jqc, jkt // 2)
                        if jkt == N_KT - 1:
                            finalize(jqc)
                return av_last

            # ============== msa + residual + LayerNorm step factories ==========
            def msa_pe_steps(k):
                """lhs load + msa matmuls, e-half-major with the residual add
                right after each half so the acc-psum slot frees quickly (the
                next shard's alloc would otherwise stall the in-order PE).
                The first matmul is ORDER-GATED (sync=False) behind b1
                attention progress: without the gate the scheduler hoists the
                msa into the attention stream where its lhs-wait blocks the
                in-order PE until the collective lands."""
                steps = []
                st = {}

                def lhs_load(k=k, st=st):
                    lhs = a2ap.tile([128, 8, 128], F8, tag="lhs")
                    st["lhs"] = lhs
                    nc.sync.dma_start(lhs[:], a2a_out[k].ap().rearrange("j p n -> p j n"))
                steps.append(lhs_load)

                for ec in (0, 1):
                    for i in range(4):
                        def mm(i=i, ec=ec, k=k, st=st):
                            if i == 0:
                                st[f"m{ec}"] = ps_acc.tile([128, 512], F32, tag="acc",
                                                           name=f"ps_m{ec}")
                            ps_m = st[f"m{ec}"]
                            mi = nc.tensor.matmul(ps_m[:], st["lhs"][:, 2 * i:2 * i + 2, :],
                                                  msa_w_sb[:, 2 * i:2 * i + 2,
                                                           512 * ec:512 * (ec + 1)],
                                                  start=(i == 0), stop=(i == 3),
                                                  perf_mode=DR)
                            if i == 0 and ec == 0 and av_anchors:
                                _add_dep_helper(
                                    mi.ins, av_anchors[3].ins, sync=False,
                                    reason="msa gated behind b1 attention end")
                        steps.append(mm)

                    def res_step(ec=ec, k=k, st=st):
                        ps_m = st[f"m{ec}"]
                        ri = nc.vector.tensor_tensor(
                            res_sb[:, k, 512 * ec:512 * (ec + 1)], ps_m[:],
                            y_shard_sb[:, k, 512 * ec:512 * (ec + 1)], ALU.add)
                        if ec == 0 and trigs:
                            # keep the DVE finalize/trigger chains ahead of
                            # the LN work (sync=True here deadlocks)
                            _add_dep_helper(
                                ri.ins, trigs[3].ins, sync=False,
                                reason="LN deprioritized behind last trigger")
                    steps.append(res_step)
                return steps

            def msa_ln_steps(k):
                steps = []

                def ln_a(k=k):
                    stats = finp.tile([128, 2, 6], F32, tag="stats")
                    nc.vector.bn_stats(stats[:, 0, :], res_sb[:, k, 0:512])
                    nc.vector.bn_stats(stats[:, 1, :], res_sb[:, k, 512:1024])
                    mu = mu_t[k]
                    nc.vector.bn_aggr(mu[:, 0:2], stats[:])
                    # res carries a WSC^2=1024 scale, so var carries 1024^2:
                    # v = var/2^20 + eps is the TRUE variance + eps, inside
                    # the rsqrt seed's fit range; negate mean for the apply
                    nc.vector.tensor_scalar(mu[:, 1:2], mu[:, 1:2], 2.0 ** -20, EPS,
                                            ALU.mult, ALU.add)
                    nc.vector.tensor_scalar(mu[:, 0:1], mu[:, 0:1], -1.0, None, ALU.mult)
                steps.append(ln_a)

                def ln_b(k=k):
                    # rstd = 1/sqrt(v) via quadratic seed + 3 Newton steps,
                    # all on DVE (keeps the ACT exp table resident)
                    mu = mu_t[k]
                    v = mu[:, 1:2]
                    y = mu[:, 3:4]
                    t = mu[:, 4:5]
                    nc.vector.tensor_scalar(y, v, RSQ_C2, RSQ_C1, ALU.mult, ALU.add)
                    nc.vector.tensor_tensor(y, y, v, ALU.mult)
                    nc.vector.tensor_scalar(y, y, RSQ_C0, None, ALU.add)
                    for _ in range(3):
                        nc.vector.tensor_tensor(t, y, y, ALU.mult)
                        nc.vector.tensor_tensor(t, t, v, ALU.mult)
                        nc.vector.tensor_scalar(t, t, -0.5, 1.5, ALU.mult, ALU.add)
                        nc.vector.tensor_tensor(y, y, t, ALU.mult)
                    # rstd for the SCALED res rows: 1/sqrt(2^20 * v)
                    nc.vector.tensor_scalar(y, y, 2.0 ** -10, None, ALU.mult)
                steps.append(ln_b)

                def ln_c(k=k):
                    mu = mu_t[k]
                    o1 = finp.tile([128, E], F32, tag="o1")
                    nc.vector.tensor_scalar(o1[:], res_sb[:, k, :], mu[:, 0:1],
                                            mu[:, 3:4], ALU.add, ALU.mult)
                    if use_gamma:
                        nc.vector.tensor_tensor(o1[:], o1[:], gamma_sb[:], ALU.mult)
                    if use_beta:
                        nc.vector.tensor_tensor(o1[:], o1[:], beta_sb[:], ALU.add)
                    nc.sync.dma_start(out[128 * k:128 * (k + 1), :], o1[:])
                steps.append(ln_c)
                return steps

            # ============== drive ==============
            q0, k0, v0 = make_qkv_steps(0)
            for s in k0[0]:
                s()
            for s in q0[0]:
                s()
            emit_late_loads()
            emit_bulk_consts()
            q1, k1, v1 = make_qkv_steps(1)
            # bgA feeds b0 qc0: k0[1] first (needed by scores step 4), then
            # v0[0] (pops at step 2+, after the scores(0,1) ordering anchor
            # exists); kT chunk c needed by step 4c, q0[1] before qc1.
            bgA = list(k0[1]) + list(v0[0]) + list(v0[1])
            for tc8 in (2, 3):
                bgA.extend(k0[tc8])
                bgA.extend(v0[tc8])
            bgA.extend(q0[1])
            attention(0, bgA, qcs=[0])
            while bgA:
                bgA.pop(0)()
            # qc1: rest of b0's Q + b1's kT (24 closures, 32 slots)
            bgB = list(q0[2]) + list(q0[3])
            for tc8 in range(4):
                bgB.extend(k1[tc8])
            attention(0, bgB, qcs=[1])
            while bgB:
                bgB.pop(0)()
            # qc2+qc3: b1's Q and V (32 closures, 32 slots)
            bgC = []
            for tc8 in range(4):
                bgC.extend(q1[tc8])
                bgC.extend(v1[tc8])
            attention(0, bgC, qcs=[2, 3], pops=1)
            while bgC:
                bgC.pop(0)()

            # b1 attention runs clean (ACT-limited; any extra PE work here
            # starves the exp stream). All msa+LN work sits after it: by then
            # collectives 0-2 have long landed regardless of the cross-core
            # launch stagger, and the ~15us of msa matmuls keep the PE p-state
            # warm across the last A2A's latency. The k=3 shard's chain is the
            # only stagger-exposed piece, same as the collective itself.
            attention(1, [], av_last=av_anchors)
            for k in range(3):
                for s in msa_pe_steps(k):
                    s()
            for k in range(3):
                for s in msa_ln_steps(k):
                    s()

            # a few warm-up dummies in case the msa work drains before the
            # last A2A lands (order-gated so the scheduler can't hoist them
            # into the attention stream)
            dmy = ps_sc.tile([128, 1024], F32, tag="sc", name="dmy")
            for i in range(12):
                di = nc.tensor.matmul(dmy[:, 0:512], wqT_sb[:, 0:2, :],
                                      msa_w_sb[:, 0:2, 0:512], start=True, stop=True,
                                      perf_mode=DR)
                if i == 0:
                    _add_dep_helper(di.ins, av_anchors[3].ins, sync=False,
                                    reason="dummies after attention")

            # last shard: msa + LN tail
            for s in msa_pe_steps(3):
                s()
            for s in msa_ln_steps(3):
                s()

    nc.compile()
    return nc


@functools.lru_cache(maxsize=4)
def _get_nc(use_gamma: bool, use_beta: bool):
    return _build(use_gamma, use_beta)


def kernel(**inputs) -> np.ndarray:
    y = np.asarray(inputs["y"], np.float32)
    Wqkv = np.asarray(inputs["Wqkv"], np.float32)
    bqkv = np.asarray(inputs["bqkv"], np.float32)
    Wmsa = np.asarray(inputs["Wmsa"], np.float32)
    Bq_, Aq_ = np.asarray(inputs["Bq"], np.float32), np.asarray(inputs["Aq"], np.float32)
    Bk_, Ak_ = np.asarray(inputs["Bk"], np.float32), np.asarray(inputs["Ak"], np.float32)
    Bv_, Av_ = np.asarray(inputs["Bv"], np.float32), np.asarray(inputs["Av"], np.float32)
    Bo_, Ao_ = np.asarray(inputs["Bo"], np.float32), np.asarray(inputs["Ao"], np.float32)
    gamma = np.asarray(inputs["gamma"], np.float32)
    beta = np.asarray(inputs["beta"], np.float32)

    # effective weights: qkv = y @ (Wqkv.T + blockdiag-ish LoRA) + bqkv
    W_eff = Wqkv.copy()
    W_eff[0:E] += (Bq_ @ Aq_).T
    W_eff[E:2 * E] += (Bk_ @ Ak_).T
    W_eff[2 * E:3 * E] += (Bv_ @ Av_).T
    # fp8 pre-scale: weights x32 (1/sqrt(D) and the scale unwind live in
    # the exp affine scale and the /32 on the msa weights)
    W_eff *= WSC
    bq_eff = bqkv[0:E] * WSC
    # K bias dropped: softmax(q.(k+bk)) == softmax(q.k) (per-row constant)
    bv_raw = bqkv[2 * E:3 * E]          # unscaled V bias, folded into y_shard
    # msa: o @ Wmsa.T + o @ (Bo@Ao) = o @ M with M = Wmsa.T + Bo@Ao  [E(d), E(out)].
    # A2A payload carries 32*o; store 32*M in fp8 so the msa psum is 1024*msa,
    # and scale the residual (y + bv@M) by 1024 to match -- LayerNorm is
    # scale-invariant, so the final output is unchanged.
    M = (Wmsa.T + Bo_ @ Ao_)
    resid_bias = bv_raw @ M             # exact: (o+bv)@M == o@M + bv@M
    M_f8 = np.ascontiguousarray(M * WSC).astype(NP_F8)

    y_flat = y.reshape(T, E)
    yT_f8 = np.ascontiguousarray(y_flat.T).astype(NP_F8)

    use_gamma = not np.allclose(gamma, 1.0)
    use_beta = not np.allclose(beta, 0.0)
    nc = _get_nc(use_gamma, use_beta)

    # column permutation for the DR-scores layout: PSUM partition p<64 holds
    # sub0 dims (h = p//32, d = p%32), p>=64 holds sub1 dims (d = p%32 + 32)
    perm = np.empty(128, np.int64)
    for p in range(128):
        h, dd = (p % 64) // 32, (p % 32) + 32 * (p // 64)
        perm[p] = h * 64 + dd

    in_maps = []
    for c in range(N_CORES):
        r0 = c * 128
        r1 = r0 + 128
        wq_c = np.ascontiguousarray(W_eff[0:E][r0:r1][perm].T).astype(NP_F8)
        wk_c = np.ascontiguousarray(W_eff[E:2 * E][r0:r1][perm].T).astype(NP_F8)
        wv_c = np.ascontiguousarray(W_eff[2 * E:3 * E][r0:r1].T).astype(NP_F8)
        tok = np.concatenate([
            np.arange(128 * c, 128 * c + 128),
            np.arange(1024 + 128 * c, 1024 + 128 * c + 128),
            np.arange(2048 + 128 * c, 2048 + 128 * c + 128),
            np.arange(3072 + 128 * c, 3072 + 128 * c + 128),
        ])
        m = {
            "yT": yT_f8,
            "wqT": wq_c,
            "wkT": wk_c,
            "wvT": wv_c,
            "bq": bq_eff[r0:r1][perm].reshape(128, 1).copy(),
            "msa_w": M_f8,
            "y_shard": (np.ascontiguousarray(y_flat[tok] + resid_bias)
                        * (WSC * WSC)).astype(ml_dtypes.bfloat16),
        }
        if use_gamma:
            m["gamma_b"] = np.broadcast_to(gamma, (128, E)).copy()
        if use_beta:
            m["beta_b"] = np.broadcast_to(beta, (128, E)).copy()
        in_maps.append(m)

    res = bass_utils.run_bass_kernel_spmd(nc, in_maps, core_ids=list(range(N_CORES)))

    out_full = np.empty((T, E), np.float32)
    for c in range(N_CORES):
        oc = res.results[c]["out"]
        out_full[128 * c:128 * c + 128] = oc[0:128]
        out_full[1024 + 128 * c:1024 + 128 * c + 128] = oc[128:256]
        out_full[2048 + 128 * c:2048 + 128 * c + 128] = oc[256:384]
        out_full[3072 + 128 * c:3072 + 128 * c + 128] = oc[384:512]
    return out_full.reshape(B, S, E)
